# revision 17
# baseline (speedup 1.0000x reference)
"""Trainium2 Bass kernel for AdaptivePrototypes (VQ codebook EMA update).

Reference computation (per problem):
    f_n   = features / max(||features||, eps)      # row-normalize (argmax-invariant)
    p_n   = prototypes / max(||prototypes||, eps)
    sims  = f_n @ p_n.T                            # [N, K]
    assign = argmax(sims, axis=1)
    sums  = segment_sum(features, assign, K)       # [K, D]
    counts= segment_sum(ones, assign, K)           # [K]
    means = sums / max(counts, 1)
    out   = where(counts>0, 0.9*p + 0.1*means, p)

Key algebraic fact: dividing each feature row by a positive scalar does not
change that row's argmax over k, so features are never normalized on device.

Distribution: data-parallel over N across 8 cores. Each core computes local
sums[K, D] + counts[K]; one AllReduce of [K+1, D] combines cores; the EMA
epilogue is replicated.

Per-core pipeline (supertiles of 512 rows = 4 partition-tiles), software
pipelined as A(s+1); B(s) so TensorE never waits on the copy engines:
  A: SWDGE cast-DMA features f32->bf16; fT via TensorE matmuls against the
     identity (regular matmul, not transpose-mode: faster + keeps the HAM
     clock warm); fT PSUM->SBUF bf16 copies split between DVE and ScalarE.
  B: simsT[K, 512] = p_nT.T @ fT (4 accumulated 512-wide matmuls); fp16
     round; transpose back via matmul with fp16 identity; batched argmax
     (3D access patterns, one op for all 4 row-tiles) with tie-splitting
     (divide onehot rows by tie count so fp16 ties cannot double-count);
     scatter sums += onehot.T @ f_bf, countsT += ones.T @ onehot.

Self-contained: hardcodes shapes; imports only concourse (on PYTHONPATH in
the runtime image) + numpy.
"""

import numpy as np

import concourse.bass as bass
import concourse.mybir as mybir
from concourse import bacc, tile
from concourse.bass_utils import run_bass_kernel_spmd

F32 = mybir.dt.float32
BF16 = mybir.dt.bfloat16
F16 = mybir.dt.float16
I32 = mybir.dt.int32
AX = mybir.AxisListType.X
ALU = mybir.AluOpType
ACT_FN = mybir.ActivationFunctionType

N = 200000
D = 512
K = 64
N_CORES = 8
N_LOC = N // N_CORES          # 25000
P = 128
ST = 512                      # supertile rows
UPDATE_RATE = 0.1


def build_nc(n_loc=N_LOC, n_cores=N_CORES):
    nc = bacc.Bacc("TRN2", target_bir_lowering=False, debug=False,
                   num_devices=n_cores)

    feats = nc.dram_tensor("features", [n_loc, D], F32, kind="ExternalInput")
    protos = nc.dram_tensor("prototypes", [K, D], F32, kind="ExternalInput")
    out_d = nc.dram_tensor("out", [K, D], F32, kind="ExternalOutput")

    n_st = (n_loc + ST - 1) // ST

    with tile.TileContext(nc) as tc:
        with (
            tc.tile_pool(name="const", bufs=1) as const,
            tc.tile_pool(name="sb_f", bufs=6) as sb_f,
            tc.tile_pool(name="sb_ft", bufs=3) as sb_ft,
            tc.tile_pool(name="sb_sims", bufs=2) as sb_sims,
            tc.tile_pool(name="sb_small", bufs=4) as sb_small,
            tc.tile_pool(name="ps_ft", bufs=2, space="PSUM") as ps_ft,
            tc.tile_pool(name="ps_sims", bufs=2, space="PSUM") as ps_sims,
            tc.tile_pool(name="ps_sn", bufs=2, space="PSUM") as ps_sn,
            tc.tile_pool(name="ps_acc", bufs=1, space="PSUM") as ps_acc,
            tc.tile_pool(name="dram", bufs=1, space="DRAM") as dram,
        ):
            # issue the first feature DMAs before anything else so the
            # SWDGE ring streams from t=0
            DMA_LEAD = 4
            dmad = {}

            def stage_dma(s):
                """Issue the cast-DMA for supertile s; returns (f_bf, meta)."""
                n0 = s * ST
                rows_st = min(ST, n_loc - n0)
                ntj = (rows_st + P - 1) // P
                rows_j = [min(P, rows_st - j * P) for j in range(ntj)]

                f_bf = sb_f.tile([P, 4 * D], BF16, name="f_bf")
                if rows_st == ST:
                    nc.gpsimd.dma_start(
                        out=f_bf[:, :].rearrange("p (t d) -> p t d", d=D),
                        in_=feats[n0:n0 + ST, :].rearrange(
                            "(t p) d -> p t d", p=P))
                else:
                    for j in range(ntj):
                        nc.gpsimd.dma_start(
                            out=f_bf[0:rows_j[j], j * D:(j + 1) * D],
                            in_=feats[n0 + j * P:n0 + j * P + rows_j[j], :])
                return f_bf, rows_st, ntj, rows_j

            for s in range(min(DMA_LEAD, n_st)):
                dmad[s] = stage_dma(s)

            # ---------------- prologue: constants ----------------
            iot = const.tile([P, P], I32, name="iot")
            nc.gpsimd.iota(iot[:, :], pattern=[[1, P]], base=0,
                           channel_multiplier=-1)
            ident_f32 = const.tile([P, P], F32, name="ident_f32")
            nc.vector.tensor_scalar(ident_f32[:, :], iot[:, :], 0, None,
                                    op0=ALU.is_equal)
            ident_bf = const.tile([P, P], BF16, name="ident_bf")
            nc.scalar.copy(ident_bf[:, :], ident_f32[:, :])
            ident_f16 = const.tile([K, K], F16, name="ident_f16")
            nc.scalar.copy(ident_f16[:, :], ident_f32[0:K, 0:K])
            ones_col = const.tile([P, 1], BF16, name="ones_col")
            nc.vector.memset(ones_col[:, :], 1.0)
            one_f32 = const.tile([1, 1], F32, name="one_f32")
            nc.vector.memset(one_f32[:, :], 1.0)

            # ---------------- prologue: prototype normalization ----------------
            p_sb = const.tile([K, D], F32, name="p_sb")
            nc.sync.dma_start(out=p_sb[:, :], in_=protos[:, :])
            psq = const.tile([K, D], F32, name="psq")
            nrm2 = const.tile([K, 1], F32, name="nrm2")
            nc.scalar.activation(psq[:, :], p_sb[:, :], ACT_FN.Square,
                                 accum_out=nrm2[:, :])
            nrm = const.tile([K, 1], F32, name="nrm")
            nc.scalar.activation(nrm[:, :], nrm2[:, :], ACT_FN.Sqrt)
            nrmc = const.tile([K, 1], F32, name="nrmc")
            nc.vector.tensor_scalar_max(nrmc[:, :], nrm[:, :], 1e-8)
            pinv = const.tile([K, 1], F32, name="pinv")
            nc.vector.reciprocal(pinv[:, :], nrmc[:, :])
            p_n_bf = const.tile([K, D], BF16, name="p_n_bf")
            nc.vector.tensor_scalar_mul(p_n_bf[:, :], p_sb[:, :], pinv[:, 0:1])

            # transpose p_n [K, D] -> pT [D, K] stored as [128, 4*K] bf16
            pT_sb = const.tile([P, 4 * K], BF16, name="pT_sb")
            for c in range(4):
                pT_ps = ps_sims.tile([P, K], BF16, name="pT_ps",
                                     tag="simsT_ps")
                nc.tensor.transpose(pT_ps[:, :],
                                    p_n_bf[0:K, c * P:(c + 1) * P],
                                    ident_bf[0:K, 0:K])
                nc.vector.tensor_copy(pT_sb[:, c * K:(c + 1) * K], pT_ps[:, :])

            # ---------------- accumulators ----------------
            sums_acc = ps_acc.tile([K, D], F32, name="sums_acc")
            countsT_acc = ps_acc.tile([1, K], F32, name="countsT_acc")

            # ---------------- main loop (software pipelined) ----------------
            def stage_trans(dmad):
                """Transpose supertile s. Returns full staged tuple."""
                f_bf, rows_st, ntj, rows_j = dmad
                fT_sb = sb_ft.tile([P, 4 * ST], BF16, name="fT_sb")
                fT_view = fT_sb[:, :].rearrange("p (c n) -> p c n", n=ST)
                for j in range(ntj):
                    r = rows_j[j]
                    fT_ps = ps_ft.tile([P, D], F32, name="fT_ps")
                    for c in range(4):
                        nc.tensor.matmul(
                            fT_ps[:, c * P:c * P + r],
                            f_bf[0:r, j * D + c * P:j * D + (c + 1) * P],
                            ident_bf[0:r, 0:r],
                            start=True, stop=True)
                    # one batched 4-chunk copy per row-tile; DVE j 0-1, ACT 2-3
                    src = fT_ps[:, :].rearrange("p (c n) -> p c n", n=P)
                    dst = fT_view[:, :, j * P:j * P + r]
                    if r < P:
                        src = src[:, :, 0:r]
                    if j < 2:
                        nc.vector.tensor_copy(dst, src)
                    else:
                        nc.scalar.copy(dst, src)
                return f_bf, fT_sb, rows_st, ntj, rows_j

            first_mm = [True]

            def stage_b1(s, staged):
                """Sims + transpose-back + argmax for supertile s."""
                f_bf, fT_sb, rows_st, ntj, rows_j = staged

                simsT_ps = ps_sims.tile([K, ST], F32, name="simsT_ps",
                                        tag="simsT_ps")
                for c in range(4):
                    nc.tensor.matmul(
                        simsT_ps[0:K, 0:rows_st],
                        pT_sb[:, c * K:(c + 1) * K],
                        fT_sb[:, c * ST:c * ST + rows_st],
                        start=(c == 0), stop=(c == 3))

                simsT_sb = sb_sims.tile([K, ST], F16, name="simsT_sb")
                nc.scalar.copy(simsT_sb[0:K, 0:rows_st],
                               simsT_ps[0:K, 0:rows_st])
                sn_ps = ps_sn.tile([P, 4 * K], F32, name="sn_ps")
                for j in range(ntj):
                    r = rows_j[j]
                    nc.tensor.matmul(
                        sn_ps[0:r, j * K:(j + 1) * K],
                        simsT_sb[0:K, j * P:j * P + r],
                        ident_f16[0:K, 0:K],
                        start=True, stop=True)

                onehot = sb_small.tile([P, 4 * K], BF16, name="onehot")
                if rows_st == ST:
                    # batched argmax + tie-splitting over all 4 row-tiles
                    sn3 = sn_ps[:, :].rearrange("p (g k) -> p g k", k=K)
                    rmax = sb_small.tile([P, 4], F32, name="rmax")
                    nc.vector.tensor_reduce(rmax[:, :], sn3, axis=AX,
                                            op=ALU.max)
                    oh_raw = sb_small.tile([P, 4 * K], F32, name="oh_raw")
                    oh3 = oh_raw[:, :].rearrange("p (g k) -> p g k", k=K)
                    nc.vector.tensor_tensor(
                        oh3, sn3,
                        rmax[:, :].unsqueeze(-1).broadcast_to([P, 4, K]),
                        op=ALU.is_equal)
                    ties = sb_small.tile([P, 4], F32, name="ties")
                    nc.vector.tensor_reduce(ties[:, :], oh3, axis=AX,
                                            op=ALU.add)
                    tinv = sb_small.tile([P, 4], F32, name="tinv")
                    nc.vector.reciprocal(tinv[:, :], ties[:, :])
                    nc.vector.tensor_tensor(
                        onehot[:, :].rearrange("p (g k) -> p g k", k=K),
                        oh3,
                        tinv[:, :].unsqueeze(-1).broadcast_to([P, 4, K]),
                        op=ALU.mult)
                else:
                    for j in range(ntj):
                        r = rows_j[j]
                        sn = sn_ps[0:r, j * K:(j + 1) * K]
                        rmax = sb_small.tile([P, 1], F32, name="rmax_t")
                        nc.vector.reduce_max(rmax[0:r, :], sn, axis=AX)
                        oh_raw = sb_small.tile([P, K], F32, name="oh_raw_t")
                        nc.vector.tensor_scalar(oh_raw[0:r, :], sn,
                                                rmax[0:r, 0:1], None,
                                                op0=ALU.is_equal)
                        ties = sb_small.tile([P, 1], F32, name="ties_t")
                        nc.vector.tensor_reduce(ties[0:r, :], oh_raw[0:r, :],
                                                axis=AX, op=ALU.add)
                        tinv = sb_small.tile([P, 1], F32, name="tinv_t")
                        nc.vector.reciprocal(tinv[0:r, :], ties[0:r, :])
                        nc.vector.tensor_scalar_mul(
                            onehot[0:r, j * K:(j + 1) * K],
                            oh_raw[0:r, :], tinv[0:r, 0:1])
                return onehot

            def stage_b2(s, staged, onehot):
                """Scatter + counts for supertile s."""
                f_bf, fT_sb, rows_st, ntj, rows_j = staged
                last_st = s == n_st - 1
                for j in range(ntj):
                    r = rows_j[j]
                    start = first_mm[0]
                    stop = last_st and j == ntj - 1
                    first_mm[0] = False
                    nc.tensor.matmul(sums_acc[:, :],
                                     onehot[0:r, j * K:(j + 1) * K],
                                     f_bf[0:r, j * D:(j + 1) * D],
                                     start=start, stop=stop)
                    nc.tensor.matmul(countsT_acc[:, :], ones_col[0:r, :],
                                     onehot[0:r, j * K:(j + 1) * K],
                                     start=start, stop=stop)

            # pipeline: DMAs lead by 4 supertiles; transposes lead by 2;
            # TRANS(s+2) is emitted between B1(s+1) and B2(s) so TensorE has
            # transpose work in flight while the DVE argmax chain completes.
            staged = {}
            staged[0] = stage_trans(dmad.pop(0))
            if n_st > 1:
                staged[1] = stage_trans(dmad.pop(1))
            oh = stage_b1(0, staged[0])
            for s in range(n_st):
                if s + DMA_LEAD < n_st:
                    dmad[s + DMA_LEAD] = stage_dma(s + DMA_LEAD)
                if s + 2 < n_st:
                    staged[s + 2] = stage_trans(dmad.pop(s + 2))
                stage_b2(s, staged[s], oh)
                if s + 1 < n_st:
                    oh = stage_b1(s + 1, staged[s + 1])
                del staged[s]

            # ---------------- epilogue: all-reduce + EMA ----------------
            sums_sb = const.tile([K, D], F32, name="sums_sb")
            nc.vector.tensor_copy(sums_sb[:, :], sums_acc[:, :])
            crow = const.tile([1, D], F32, name="crow")
            nc.vector.memset(crow[:, :], 0.0)
            nc.vector.tensor_copy(crow[0:1, 0:K], countsT_acc[:, :])

            # All-gather via AllToAll of an 8x-replicated input: AllToAll is
            # pairwise (no ring/RDH staging), so it has the lowest ncfw
            # latency at this size. Each rank sends a copy of its slab to
            # every rank; the received concatenation is the all-gather.
            cc_in = dram.tile([K + 1, D], F32, name="cc_in")
            cc_rep = dram.tile([(K + 1) * n_cores, D], F32, name="cc_rep")
            cc_ag = dram.tile([(K + 1) * n_cores, D], F32, name="cc_ag")
            nc.sync.dma_start(out=cc_in[0:K, :], in_=sums_sb[:, :])
            nc.sync.dma_start(out=cc_in[K:K + 1, :], in_=crow[:, :])
            nc.sync.dma_start(
                out=cc_rep[:, :].rearrange("(r p) d -> r p d", r=n_cores),
                in_=cc_in[:, :].unsqueeze(0).broadcast_to(
                    [n_cores, K + 1, D]))
            nc.gpsimd.collective_compute(
                "AllToAll", ALU.bypass,
                replica_groups=[list(range(n_cores))],
                ins=[cc_rep.opt()], outs=[cc_ag.opt()])

            # [(r, p), d] -> SBUF [K+1, (d, r)]; reduce ranks in one 3D op
            # with r innermost (sums rows base 0 and the counts row
            # separately, to keep partition bases aligned)
            ag_sb = const.tile([K + 1, n_cores * D], F32, name="ag_sb")
            nc.sync.dma_start(
                out=ag_sb[:, :].rearrange("p (r d) -> p r d", d=D),
                in_=cc_ag[:, :].rearrange("(r p) d -> p r d", p=K + 1))
            w = n_cores * D
            while w > D:
                h = w // 2
                nc.vector.tensor_add(ag_sb[0:K, 0:h], ag_sb[0:K, 0:h],
                                     ag_sb[0:K, h:w])
                nc.vector.tensor_add(ag_sb[K:K + 1, 0:h],
                                     ag_sb[K:K + 1, 0:h],
                                     ag_sb[K:K + 1, h:w])
                w = h
            red = ag_sb
            # counts live on partition K; move to partition 0 for the matmul
            ar_cnt = const.tile([1, K], F32, name="ar_cnt")
            nc.sync.dma_start(out=ar_cnt[:, :], in_=red[K:K + 1, 0:K])

            # counts [1, K] -> [K, 1] via tiny matmul against ones [1, 1]
            cnt_ps = ps_sn.tile([K, 1], F32, name="cnt_ps", tag="sn_ps")
            nc.tensor.matmul(cnt_ps[:, :], ar_cnt[:, :],
                             one_f32[:, :], start=True, stop=True)
            cnt = const.tile([K, 1], F32, name="cnt")
            nc.vector.tensor_copy(cnt[:, :], cnt_ps[:, :])

            cntc = const.tile([K, 1], F32, name="cntc")
            nc.vector.tensor_scalar_max(cntc[:, :], cnt[:, :], 1.0)
            rcp = const.tile([K, 1], F32, name="rcp")
            nc.vector.reciprocal(rcp[:, :], cntc[:, :])
            means = const.tile([K, D], F32, name="means")
            nc.vector.tensor_scalar_mul(means[:, :], red[0:K, 0:D],
                                        rcp[:, 0:1])
            diff = const.tile([K, D], F32, name="diff")
            nc.vector.tensor_sub(diff[:, :], means[:, :], p_sb[:, :])
            mask = const.tile([K, 1], F32, name="mask")
            nc.vector.tensor_scalar(mask[:, :], cnt[:, :], 0.0, None,
                                    op0=ALU.is_gt)
            scale = const.tile([K, 1], F32, name="scale")
            nc.vector.tensor_scalar_mul(scale[:, :], mask[:, :], UPDATE_RATE)
            out_sb = const.tile([K, D], F32, name="out_sb")
            nc.vector.scalar_tensor_tensor(out_sb[:, :], diff[:, :],
                                           scale[:, 0:1], p_sb[:, :],
                                           op0=ALU.mult, op1=ALU.add)
            nc.sync.dma_start(out=out_d[:, :], in_=out_sb[:, :])

    nc.compile()
    return nc


_NC_CACHE = {}


def _get_nc(n_loc=N_LOC, n_cores=N_CORES):
    key = (n_loc, n_cores)
    if key not in _NC_CACHE:
        _NC_CACHE[key] = build_nc(n_loc, n_cores)
    return _NC_CACHE[key]


def run(features, prototypes, trace=False, **kwargs):
    """Run on hardware; returns (output, BassKernelResults)."""
    features = np.ascontiguousarray(features, dtype=np.float32)
    prototypes = np.ascontiguousarray(prototypes, dtype=np.float32)
    assert features.shape == (N, D) and prototypes.shape == (K, D)
    nc = _get_nc()
    shards = np.split(features, N_CORES, axis=0)
    in_maps = [{"features": np.ascontiguousarray(s), "prototypes": prototypes}
               for s in shards]
    res = run_bass_kernel_spmd(nc, in_maps, core_ids=list(range(N_CORES)),
                               trace=trace, **kwargs)
    return res.results[0]["out"], res


def kernel(features, prototypes):
    out, _ = run(features, prototypes)
    return out
